# revision 1
# baseline (speedup 1.0000x reference)
"""TRN2 Bass kernel for nn_Der_SRec: attention-fused embedding scorer.

Math (per row b of batch B=16384, D=512):
  z,c,f = Ez[n[b]], Ec[n[b]], E[n[b]]       (per side u/v)
  s_z = a3 . relu(A2 @ relu(A1a @ z + A1f @ f + ab1) + ab2) + ab3
  s_c = same with c
  w_z = softmax([s_z, s_c])[0] = sigmoid(s_z - s_c)   (ab3 cancels)
  u = c + w_z * (z - c)
  h  = relu(bn(uv @ w1.T + b1));  out = h @ w2.T + b2  (bn folded into w1/b1)

Distribution: data-parallel over batch across 8 cores (2048 rows/core);
tables + weights replicated. On-chip: activations live in [feature, batch]
layout (feature on partitions) so the TensorE contracts features; the gather
produces [batch, feature] so each gathered tile is PE-transposed. The
indirect gather casts f32 tables to bf16 in the DMA; all matmuls run in bf16
with f32 PSUM accumulation; the shared `full`-conditioned first-layer term is
computed once per side and added to both scores' PSUM.
"""
import numpy as np
import ml_dtypes

import concourse.bass as bass
import concourse.mybir as mybir
import concourse.tile as tile
from concourse.bass_utils import run_bass_kernel_spmd
from concourse.masks import make_identity

P = 128
D = 512
DC = D // P          # feature chunks per 512
B = 16384
NCORES = 8
BC = B // NCORES     # rows per core (2048)
BT = 512             # batch tile (matmul N)
NBT = BC // BT       # batch tiles per core (4)
NSUB = BT // P       # gather subtiles per batch tile (4)
NU = 100000
NV = 50000
BN_EPS = 1e-5

f32 = mybir.dt.float32
bf16 = mybir.dt.bfloat16
i32 = mybir.dt.int32

_uid = [0]


def _split_multi_waits(nc):
    """walrus here encodes at most ONE sem wait per ISA instruction; Tile's
    sem assignment can emit several on one instruction (kernel-tail drain,
    matmuls with several producers). Hoist extras onto single-wait NoOps
    inserted just before, on the same engine stream (same-engine program
    order preserves semantics)."""
    for fn in nc.m.functions:
        for blk in fn.blocks:
            insts = blk.instructions
            i = 0
            while i < len(insts):
                inst = insts[i]
                si = inst.sync_info
                if si is not None and len(si.on_wait) > 1:
                    waits = list(si.on_wait)
                    for w in waits[:-1]:
                        _uid[0] += 1
                        nop = mybir.InstNoOp(
                            name=f"waitsplit_{_uid[0]}", ins=[], outs=[]
                        )
                        nop.engine = inst.engine
                        nop.sync_info = mybir.SyncInfo(on_wait=[w], on_update=[])
                        insts.insert(i, nop)
                        i += 1
                    inst.sync_info = mybir.SyncInfo(
                        on_wait=[waits[-1]], on_update=list(si.on_update)
                    )
                i += 1


def _build():
    nc = bass.Bass()

    tab_u = {
        "z": nc.dram_tensor("Ez_u", [NU, D], f32, kind="ExternalInput"),
        "c": nc.dram_tensor("Ec_u", [NU, D], f32, kind="ExternalInput"),
        "f": nc.dram_tensor("E_u", [NU, D], f32, kind="ExternalInput"),
    }
    tab_v = {
        "z": nc.dram_tensor("Ez_v", [NV, D], f32, kind="ExternalInput"),
        "c": nc.dram_tensor("Ec_v", [NV, D], f32, kind="ExternalInput"),
        "f": nc.dram_tensor("E_v", [NV, D], f32, kind="ExternalInput"),
    }
    nodes_u = nc.dram_tensor("nodes_u", [BC], i32, kind="ExternalInput")
    nodes_v = nc.dram_tensor("nodes_v", [BC], i32, kind="ExternalInput")

    # weight layout: [D_in, X] row-major in DRAM, loaded as [p, kc, X] in SBUF
    A1aT = nc.dram_tensor("A1aT", [D, D], bf16, kind="ExternalInput")
    A1fT = nc.dram_tensor("A1fT", [D, D], bf16, kind="ExternalInput")
    A2T = nc.dram_tensor("A2T", [D, D], bf16, kind="ExternalInput")
    W1uT = nc.dram_tensor("W1uT", [D, D], bf16, kind="ExternalInput")
    W1vT = nc.dram_tensor("W1vT", [D, D], bf16, kind="ExternalInput")
    a3p = nc.dram_tensor("a3p", [D], bf16, kind="ExternalInput")
    w2T = nc.dram_tensor("w2T", [D], bf16, kind="ExternalInput")
    ab1 = nc.dram_tensor("ab1", [DC, P], f32, kind="ExternalInput")
    ab2 = nc.dram_tensor("ab2", [DC, P], f32, kind="ExternalInput")
    bh = nc.dram_tensor("bh", [DC, P], f32, kind="ExternalInput")

    out = nc.dram_tensor("out", [BC], f32, kind="ExternalOutput")

    with tile.TileContext(nc) as tc:
        with (
            tc.tile_pool(name="const", bufs=1) as const,
            tc.tile_pool(name="rawp", bufs=30) as rawp,
            tc.tile_pool(name="xp", bufs=2) as xp,
            tc.tile_pool(name="hp", bufs=2) as hp,
            tc.tile_pool(name="sp", bufs=2) as sp,
            tc.tile_pool(name="ps_tr", bufs=2, space="PSUM") as ps_tr,
            tc.tile_pool(name="ps_mm", bufs=5, space="PSUM") as ps_mm,
            tc.tile_pool(name="ps_aux", bufs=1, space="PSUM") as ps_aux,
        ):
            ident = const.tile([P, P], bf16)
            make_identity(nc, ident)
            ones_bc = const.tile([1, P], bf16)
            nc.vector.memset(ones_bc[:], 1.0)

            # bt0 index columns first (unblocks the first gathers), on two
            # different HWDGE queues; the rest loads behind them.
            idx_u = const.tile([P, BC // P], i32)
            idx_v = const.tile([P, BC // P], i32)
            nodes_u_pt = nodes_u[:].rearrange("(t p) -> p t", p=P)
            nodes_v_pt = nodes_v[:].rearrange("(t p) -> p t", p=P)
            nc.sync.dma_start(out=idx_u[:, 0:NSUB], in_=nodes_u_pt[:, 0:NSUB])
            nc.scalar.dma_start(out=idx_v[:, 0:NSUB], in_=nodes_v_pt[:, 0:NSUB])
            nc.sync.dma_start(out=idx_u[:, NSUB:], in_=nodes_u_pt[:, NSUB:])
            nc.scalar.dma_start(out=idx_v[:, NSUB:], in_=nodes_v_pt[:, NSUB:])


            def load_w(dram):
                t = const.tile([P, DC, D], bf16, name=f"w_{dram.name}")
                nc.sync.dma_start(
                    out=t[:], in_=dram[:, :].rearrange("(kc p) m -> p kc m", p=P)
                )
                return t

            A1aT_sb = load_w(A1aT)
            A1fT_sb = load_w(A1fT)
            A2T_sb = load_w(A2T)
            W1uT_sb = load_w(W1uT)
            W1vT_sb = load_w(W1vT)

            def load_vec(dram, dt):
                t = const.tile([P, DC], dt, name=f"v_{dram.name}")
                nc.sync.dma_start(
                    out=t[:], in_=dram[:].rearrange("(kc p) -> p kc", p=P)
                )
                return t

            a3p_sb = load_vec(a3p, bf16)
            w2T_sb = load_vec(w2T, bf16)

            def load_bias(dram):
                t = const.tile([P, DC], f32, name=f"b_{dram.name}")
                nc.sync.dma_start(
                    out=t[:], in_=dram[:, :].rearrange("kc p -> p kc")
                )
                return t

            ab1_sb = load_bias(ab1)
            ab2_sb = load_bias(ab2)
            bh_sb = load_bias(bh)

            def stage_gather(bt):
                """Issue the 24 indirect row-gathers for batch tile bt."""
                raws = {}
                for side, tabs, idx in (("u", tab_u, idx_u), ("v", tab_v, idx_v)):
                    for kind in ("z", "c", "f"):
                        rs = []
                        for s in range(NSUB):
                            raw = rawp.tile(
                                [P, D], bf16, name=f"raw_{side}{kind}{s}", tag="raw"
                            )
                            nc.gpsimd.indirect_dma_start(
                                out=raw[:],
                                out_offset=None,
                                in_=tabs[kind][:],
                                in_offset=bass.IndirectOffsetOnAxis(
                                    ap=idx[:, bt * NSUB + s : bt * NSUB + s + 1],
                                    axis=0,
                                ),
                            )
                            rs.append(raw)
                        raws[(side, kind)] = rs
                return raws

            def stage_transpose(raws):
                """PE-transpose gathered [batch, feat] tiles into [feat, batch]."""
                xT = {}
                for key, rs in raws.items():
                    side, kind = key
                    x = xp.tile(
                        [P, DC, BT], bf16, name=f"xT_{side}{kind}",
                        tag=f"xT_{side}{kind}",
                    )
                    for c in range(DC):
                        pst = ps_tr.tile(
                            [P, BT], bf16, name=f"pst{c}", tag="pst"
                        )
                        for s in range(NSUB):
                            nc.tensor.transpose(
                                pst[:, s * P : (s + 1) * P],
                                rs[s][:, c * P : (c + 1) * P],
                                ident[:],
                            )
                        nc.any.tensor_copy(x[:, c, :], pst[:])
                    xT[key] = x
                return xT

            raws_cur = stage_gather(0)
            for bt in range(NBT):
                xT = stage_transpose(raws_cur)
                if bt + 1 < NBT:
                    raws_cur = stage_gather(bt + 1)

                # ---- per-side attention fusion -> u_t, v_t bf16 [p, kc, BT]
                fused = {}
                for side in ("u", "v"):
                    xz, xc, xf = (
                        xT[(side, "z")], xT[(side, "c")], xT[(side, "f")],
                    )

                    def mlp_layer(wa, xa, bias_sb, name, add_sb=None):
                        h = hp.tile(
                            [P, DC, BT], bf16, name=f"h_{name}", tag=f"h_{name}"
                        )
                        for m in range(DC):
                            ps = ps_mm.tile(
                                [P, BT], f32, name=f"ps_{name}{m}", tag="mm"
                            )
                            for k in range(DC):
                                nc.tensor.matmul(
                                    ps[:],
                                    wa[:, k, m * P : (m + 1) * P],
                                    xa[:, k, :],
                                    start=(k == 0),
                                    stop=(k == DC - 1),
                                )
                            if add_sb is not None:
                                nc.vector.tensor_add(ps[:], ps[:], add_sb[:, m, :])
                            nc.scalar.activation(
                                out=h[:, m, :],
                                in_=ps[:],
                                func=mybir.ActivationFunctionType.Relu,
                                bias=bias_sb[:, m : m + 1],
                                scale=1.0,
                            )
                        return h

                    # shared first-layer term from `full`: hf = A1f.T @ f
                    hf = hp.tile([P, DC, BT], f32, name=f"hf_{side}", tag="hf")
                    for m in range(DC):
                        ps = ps_mm.tile([P, BT], f32, name=f"ps_hf{m}", tag="mm")
                        for k in range(DC):
                            nc.tensor.matmul(
                                ps[:],
                                A1fT_sb[:, k, m * P : (m + 1) * P],
                                xf[:, k, :],
                                start=(k == 0),
                                stop=(k == DC - 1),
                            )
                        nc.vector.tensor_copy(hf[:, m, :], ps[:])

                    h1z = mlp_layer(A1aT_sb, xz, ab1_sb, "1z", add_sb=hf)
                    h1c = mlp_layer(A1aT_sb, xc, ab1_sb, "1c", add_sb=hf)
                    h2z = mlp_layer(A2T_sb, h1z, ab2_sb, "2z")
                    h2c = mlp_layer(A2T_sb, h1c, ab2_sb, "2c")

                    # d = s_z - s_c = a3 . (h2z - h2c)  [1, BT]
                    hd = hp.tile([P, DC, BT], bf16, name=f"hd_{side}", tag="hd")
                    dps = ps_aux.tile([1, BT], f32, name="dps", tag="aux")
                    # per-chunk so each L3 matmul starts as soon as its h2
                    # chunk's relu lands (overlaps L2's tail)
                    for k in range(DC):
                        nc.vector.tensor_sub(hd[:, k, :], h2z[:, k, :], h2c[:, k, :])
                        nc.tensor.matmul(
                            dps[:],
                            a3p_sb[:, k : k + 1],
                            hd[:, k, :],
                            start=(k == 0),
                            stop=(k == DC - 1),
                        )
                    wz = sp.tile([1, BT], bf16, name="wz", tag="wz")
                    nc.scalar.activation(
                        out=wz[:],
                        in_=dps[:],
                        func=mybir.ActivationFunctionType.Sigmoid,
                    )
                    # broadcast wz across partitions via K=1 ones-matmul
                    wbc = ps_aux.tile([P, BT], f32, name="wbc", tag="aux")
                    nc.tensor.matmul(
                        wbc[:], ones_bc[:], wz[:], start=True, stop=True
                    )
                    # fused = c + wz * (z - c)
                    zmc = hp.tile(
                        [P, DC, BT], bf16, name=f"zmc_{side}", tag="zmc"
                    )
                    nc.vector.tensor_sub(zmc[:], xz[:], xc[:])
                    uf = hp.tile(
                        [P, DC, BT], bf16, name=f"fused_{side}", tag=f"fused_{side}"
                    )
                    # per-chunk so head matmuls can start on early chunks
                    for k in range(DC):
                        nc.vector.tensor_tensor(
                            out=zmc[:, k, :], in0=zmc[:, k, :], in1=wbc[:],
                            op=mybir.AluOpType.mult,
                        )
                        nc.vector.tensor_add(uf[:, k, :], zmc[:, k, :], xc[:, k, :])
                    fused[side] = uf

                # ---- head: h = relu(W1u.T@u + W1v.T@v + bh) ; out = w2.h + b2
                hh = hp.tile([P, DC, BT], bf16, name="hh", tag="hh")
                for m in range(DC):
                    ps = ps_mm.tile([P, BT], f32, name=f"ps_hh{m}", tag="mm")
                    for k in range(DC):
                        nc.tensor.matmul(
                            ps[:],
                            W1uT_sb[:, k, m * P : (m + 1) * P],
                            fused["u"][:, k, :],
                            start=(k == 0),
                            stop=False,
                        )
                    for k in range(DC):
                        nc.tensor.matmul(
                            ps[:],
                            W1vT_sb[:, k, m * P : (m + 1) * P],
                            fused["v"][:, k, :],
                            start=False,
                            stop=(k == DC - 1),
                        )
                    nc.scalar.activation(
                        out=hh[:, m, :],
                        in_=ps[:],
                        func=mybir.ActivationFunctionType.Relu,
                        bias=bh_sb[:, m : m + 1],
                        scale=1.0,
                    )
                ops = ps_aux.tile([1, BT], f32, name="ops", tag="aux")
                for k in range(DC):
                    nc.tensor.matmul(
                        ops[:],
                        w2T_sb[:, k : k + 1],
                        hh[:, k, :],
                        start=(k == 0),
                        stop=(k == DC - 1),
                    )
                osb = sp.tile([1, BT], f32, name="osb", tag="osb")
                nc.scalar.activation(
                    out=osb[:],
                    in_=ops[:],
                    func=mybir.ActivationFunctionType.Copy,
                )
                nc.sync.dma_start(
                    out=out[bt * BT : (bt + 1) * BT].unsqueeze(0), in_=osb[:]
                )

    _split_multi_waits(nc)
    return nc


_NC_CACHE = None


def _get_nc():
    global _NC_CACHE
    if _NC_CACHE is None:
        _NC_CACHE = _build()
    return _NC_CACHE


def _prep_host(inputs):
    """Host-side weight preprocessing shared by all cores."""
    f = lambda k: np.asarray(inputs[k], np.float32)
    att_w1 = f("att_w1")
    att_w2 = f("att_w2")
    att_w3 = f("att_w3")
    w1 = f("w1")
    s = f("bn_gamma") / np.sqrt(f("bn_var") + BN_EPS)
    t = f("bn_beta") - f("bn_mean") * s
    bf = lambda a: np.ascontiguousarray(a).astype(ml_dtypes.bfloat16)
    common = {
        "Ez_u": f("Ez_u"), "Ec_u": f("Ec_u"), "E_u": f("E_u"),
        "Ez_v": f("Ez_v"), "Ec_v": f("Ec_v"), "E_v": f("E_v"),
        "A1aT": bf(att_w1[:, :D].T),
        "A1fT": bf(att_w1[:, D:].T),
        "A2T": bf(att_w2.T),
        "W1uT": bf((w1[:, :D] * s[:, None]).T),
        "W1vT": bf((w1[:, D:] * s[:, None]).T),
        "a3p": bf(att_w3[0]),
        "w2T": bf(f("w2")[0]),
        "ab1": np.ascontiguousarray(f("att_b1").reshape(DC, P)),
        "ab2": np.ascontiguousarray(f("att_b2").reshape(DC, P)),
        "bh": np.ascontiguousarray((f("b1") * s + t).reshape(DC, P)),
    }
    return common


def kernel(**inputs):
    common = _prep_host(inputs)
    nodes_u = np.asarray(inputs["nodes_u"]).astype(np.int32)
    nodes_v = np.asarray(inputs["nodes_v"]).astype(np.int32)

    in_maps = []
    for i in range(NCORES):
        m = dict(common)
        m["nodes_u"] = np.ascontiguousarray(nodes_u[i * BC : (i + 1) * BC])
        m["nodes_v"] = np.ascontiguousarray(nodes_v[i * BC : (i + 1) * BC])
        in_maps.append(m)

    nc = _get_nc()
    res = run_bass_kernel_spmd(nc, in_maps, core_ids=list(range(NCORES)))
    out = np.concatenate([np.asarray(r["out"]) for r in res.results])
    return (out + np.float32(np.asarray(inputs["b2"]).reshape(-1)[0])).astype(np.float32)



# revision 32
# speedup vs baseline: 1.7762x; 1.7762x over previous
"""TRN2 Bass kernel for nn_Der_SRec: attention-fused embedding scorer.

Math (per row b of batch B=16384, D=512):
  z,c,f = Ez[n[b]], Ec[n[b]], E[n[b]]       (per side u/v)
  s_z = a3 . relu(A2 @ relu(A1a @ z + A1f @ f)), s_c = same with c
  w_z = sigmoid(s_z - s_c)
  u = w_z * z + (1 - w_z) * c
  h = relu(bn(uv @ w1.T)); out = h @ w2.T + b2  (bn folded into w1)
  (all bias vectors in this problem are structurally zero - asserted on the
  host - so they are dropped from the device kernel)

Distribution: data-parallel over batch across 8 cores (2048 rows/core);
tables + weights replicated.

Implementation notes:
  - Tables pre-cast to bf16 on the host; one merged 512-row indirect gather
    per (side, tensor) per batch tile.
  - Attention MLP in fp8 (e4m3) with DoubleRow matmuls (half the PE cycles
    per instruction, 256-deep contraction via [K,2,*] APs). The z-/c-branch
    first layers fold the `full`-term in by contracting K=1024 over [x; f].
  - Fusion u = wz*z + wn*c (wn = sigmoid(-d)) in the raw [batch, feature]
    domain where wz/wn are per-partition scalars; only the fused result is
    PE-transposed for the bf16 head.
  - Score and output reductions are batch-on-partition matvecs (output free
    size 1 -> nearly free on PE); h2z-h2c folds into +a3/-a3 accumulation.
  - PSUM drains (the relus and transpose-copies) are the second-busiest
    resource after the PE and only Act/DVE can touch PSUM, so drains are
    merged into [128,1024] instructions: matmul groups accumulate into
    two-bank [P,2,BT] f32 PSUM tiles drained by a single relu; transposes
    write [P,2,BT] bf16 one-bank tiles drained by a single (casting) copy.
"""
import numpy as np
import ml_dtypes

import concourse.bass as bass
import concourse.mybir as mybir
import concourse.tile as tile
from concourse.bass_utils import run_bass_kernel_spmd
from concourse.masks import make_identity

P = 128
D = 512
DC = D // P          # feature chunks per 512
B = 16384
NCORES = 8
BC = B // NCORES     # rows per core (2048)
BT = 512             # batch tile (matmul N)
NBT = BC // BT       # batch tiles per core (4)
NSUB = BT // P       # gather subtiles per batch tile (4)
NU = 100000
NV = 50000
BN_EPS = 1e-5

f32 = mybir.dt.float32
bf16 = mybir.dt.bfloat16
fp8 = mybir.dt.float8e4
i32 = mybir.dt.int32
DR = mybir.MatmulPerfMode.DoubleRow
AF = mybir.ActivationFunctionType
ALU = mybir.AluOpType

_uid = [0]


def _split_multi_waits(nc):
    """walrus here encodes at most ONE sem wait per ISA instruction; Tile's
    sem assignment can emit several on one instruction (kernel-tail drain,
    matmuls with several producers). Hoist extras onto single-wait NoOps
    inserted just before, on the same engine stream (same-engine program
    order preserves semantics)."""
    for fn in nc.m.functions:
        for blk in fn.blocks:
            insts = blk.instructions
            i = 0
            while i < len(insts):
                inst = insts[i]
                si = inst.sync_info
                if si is not None and len(si.on_wait) > 1:
                    waits = list(si.on_wait)
                    for w in waits[:-1]:
                        _uid[0] += 1
                        nop = mybir.InstNoOp(
                            name=f"waitsplit_{_uid[0]}", ins=[], outs=[]
                        )
                        nop.engine = inst.engine
                        nop.sync_info = mybir.SyncInfo(on_wait=[w], on_update=[])
                        insts.insert(i, nop)
                        i += 1
                    inst.sync_info = mybir.SyncInfo(
                        on_wait=[waits[-1]], on_update=list(si.on_update)
                    )
                i += 1


def _build():
    nc = bass.Bass()

    # z/f/c tables concatenated host-side so one indirect DMA per side
    # gathers all three embeddings (indices pre-offset by table block).
    tab3_u = nc.dram_tensor("EU3", [3 * NU, D], bf16, kind="ExternalInput")
    tab3_v = nc.dram_tensor("EV3", [3 * NV, D], bf16, kind="ExternalInput")
    # idx3: [P, 3*(BC/P)] i32 pre-arranged host-side: [p, k*16+t] =
    # nodes[t*P+p] + k*N  (k: 0=z, 1=f, 2=c)
    idx3_u_d = nc.dram_tensor("idx3_u", [P, 3 * (BC // P)], i32, kind="ExternalInput")
    idx3_v_d = nc.dram_tensor("idx3_v", [P, 3 * (BC // P)], i32, kind="ExternalInput")

    # L1 weights stacked [A1a; A1f; A1a] so the z-branch contracts kc pairs
    # at offsets (0,2,4,6) over [z; f] and the c-branch pairs (4,6,8,10)
    # over [f; c].
    WL1 = nc.dram_tensor("WL1", [3 * D, D], fp8, kind="ExternalInput")
    A2T8 = nc.dram_tensor("A2T8", [D, D], fp8, kind="ExternalInput")
    W1uT = nc.dram_tensor("W1uT", [D, D], bf16, kind="ExternalInput")
    W1vT = nc.dram_tensor("W1vT", [D, D], bf16, kind="ExternalInput")
    a3p = nc.dram_tensor("a3p", [D], bf16, kind="ExternalInput")
    a3n = nc.dram_tensor("a3n", [D], bf16, kind="ExternalInput")
    w2T = nc.dram_tensor("w2T", [D], bf16, kind="ExternalInput")

    out = nc.dram_tensor("out", [BC], f32, kind="ExternalOutput")

    with tile.TileContext(nc) as tc:
        with (
            tc.tile_pool(name="const", bufs=1) as const,
            tc.tile_pool(name="rawp", bufs=2) as rawp,
            tc.tile_pool(name="xp", bufs=2) as xp,
            tc.tile_pool(name="hp", bufs=2) as hp,
            tc.tile_pool(name="sp", bufs=2) as sp,
            tc.tile_pool(name="ps_tr", bufs=3, space="PSUM") as ps_tr,
            tc.tile_pool(name="ps_mm", bufs=4, space="PSUM") as ps_mm,
            tc.tile_pool(name="ps_aux", bufs=1, space="PSUM") as ps_aux,
        ):
            ident = const.tile([P, P], bf16)
            make_identity(nc, ident)

            # index columns for bt0 first (unblocks the first gathers), on
            # two different HWDGE queues; the rest loads behind them.
            idx_u = const.tile([P, 3, BC // P], i32)
            idx_v = const.tile([P, 3, BC // P], i32)
            nodes_u_pt = idx3_u_d[:, :].rearrange("p (k t) -> p k t", k=3)
            nodes_v_pt = idx3_v_d[:, :].rearrange("p (k t) -> p k t", k=3)
            nc.sync.dma_start(out=idx_u[:, :, 0:NSUB], in_=nodes_u_pt[:, :, 0:NSUB])
            nc.scalar.dma_start(out=idx_v[:, :, 0:NSUB], in_=nodes_v_pt[:, :, 0:NSUB])
            nc.sync.dma_start(out=idx_u[:, :, NSUB:], in_=nodes_u_pt[:, :, NSUB:])
            nc.scalar.dma_start(out=idx_v[:, :, NSUB:], in_=nodes_v_pt[:, :, NSUB:])

            def load_w(dram, kc, dt, eng=None):
                t = const.tile([P, kc, D], dt, name=f"w_{dram.name}")
                (eng or nc.sync).dma_start(
                    out=t[:], in_=dram[:, :].rearrange("(kc p) m -> p kc m", p=P)
                )
                return t

            def load_vec(dram, dt):
                t = const.tile([P, DC], dt, name=f"v_{dram.name}")
                nc.scalar.dma_start(
                    out=t[:], in_=dram[:].rearrange("(kc p) -> p kc", p=P)
                )
                return t

            # attention weights first (needed earliest)
            WL1_sb = load_w(WL1, 3 * DC, fp8)
            A2T8_sb = load_w(A2T8, DC, fp8)
            a3p_sb = load_vec(a3p, bf16)
            a3n_sb = load_vec(a3n, bf16)
            w2T_sb = load_vec(w2T, bf16)

            KI = {"z": 0, "f": 1, "c": 2}

            def stage_gather(bt):
                """One merged 1536-row indirect gather per side (z+f+c)."""
                raws = {}
                for side, tab3, idx in (("u", tab3_u, idx_u), ("v", tab3_v, idx_v)):
                    raw3 = rawp.tile(
                        [P, 3 * NSUB, D], bf16,
                        name=f"raw3_{side}", tag=f"raw3_{side}",
                    )
                    nc.gpsimd.indirect_dma_start(
                        out=raw3[:],
                        out_offset=None,
                        in_=tab3[:],
                        in_offset=bass.IndirectOffsetOnAxis(
                            ap=idx[:, :, bt * NSUB : (bt + 1) * NSUB],
                            axis=0,
                        ),
                    )
                    raws[side] = raw3
                return raws

            raws_cur = stage_gather(0)
            # head weights load behind the first gathers (needed only late)
            W1uT_sb = load_w(W1uT, DC, bf16)
            W1vT_sb = load_w(W1vT, DC, bf16, eng=nc.scalar)

            # --- merged PSUM drains, alternating Act / DVE (the only two
            # engines with a PSUM port on TRN2) ---
            di = [0]
            DPAT = "ADADADADADA"  # ~55% Act: DVE ops cost ~15% more and also fuse

            def drain(dst, src, relu):
                use_act = DPAT[di[0] % len(DPAT)] == "A"
                di[0] += 1
                if use_act:
                    nc.scalar.activation(
                        out=dst, in_=src, func=(AF.Relu if relu else AF.Copy)
                    )
                elif relu:
                    nc.vector.tensor_scalar(
                        out=dst, in0=src, scalar1=0.0, scalar2=None, op0=ALU.max
                    )
                else:
                    nc.vector.tensor_copy(dst, src)

            def transpose_into(rawsl, dst, relu=False):
                """PE-transpose subtile chunks rawsl(s, c) -> [P, P] (bf16)
                into dst [P, DC, BT], casting to dst's dtype in merged
                [128, 2*BT] drain copies."""
                for half in range(2):
                    pst = ps_tr.tile([P, 2, BT], bf16, name="pst", tag="pst")
                    for ci in range(2):
                        c_ = 2 * half + ci
                        for s in range(NSUB):
                            nc.tensor.transpose(
                                pst[:, ci, s * P : (s + 1) * P],
                                rawsl(s, c_),
                                ident[:],
                            )
                    drain(dst[:, 2 * half : 2 * half + 2, :], pst[:], relu)

            def transpose_group(rawsl, dst, half):
                """One pst group: 8 PE transposes + 1 merged drain."""
                pst = ps_tr.tile([P, 2, BT], bf16, name="pst", tag="pst")
                for ci in range(2):
                    c_ = 2 * half + ci
                    for s in range(NSUB):
                        nc.tensor.transpose(
                            pst[:, ci, s * P : (s + 1) * P],
                            rawsl(s, c_),
                            ident[:],
                        )
                drain(dst[:, 2 * half : 2 * half + 2, :], pst[:], False)

            def filler_units(raws):
                """Allocate next tile's fp8 tiles; yield one closure per pst
                group so the caller can interleave them with other PE work."""
                xts = {}
                units = []
                for side in ("u", "v"):
                    raw3 = raws[side]
                    for kind in ("z", "f", "c"):
                        k = KI[kind]
                        xt = xp.tile(
                            [P, DC, BT], fp8, name=f"x8_{side}{kind}",
                            tag=f"x8_{side}{kind}",
                        )
                        sl = lambda s, c, k=k, r=raw3: r[:, k * NSUB + s, c * P : (c + 1) * P]
                        for half in range(2):
                            units.append(
                                lambda sl=sl, xt=xt, half=half: transpose_group(sl, xt, half)
                            )
                        xts[(side, kind)] = xt
                return xts, units

            def do_transposes(raws):
                xts, units = filler_units(raws)
                for u_ in units:
                    u_()
                return xts

            xts_by_bt = {0: do_transposes(raws_cur)}
            raws_by_bt = {0: raws_cur}
            ufs_by_bt = {}

            h2s_by_bt = {}
            fillers_by_bt = {}

            def phaseA1(bt):
                """L1+L2 for tile bt; prefetches tile bt+1's gathers and
                interleaves 8 of its 12 filler transpose groups into the L1
                window (the L2 window's drains already saturate Act+DVE)."""
                fillers = []
                if bt + 1 < NBT:
                    raws_by_bt[bt + 1] = stage_gather(bt + 1)
                    xts_by_bt[bt + 1], fillers = filler_units(raws_by_bt[bt + 1])
                fillers_by_bt[bt + 1] = fillers
                fpos = [0]
                mmg = [0]

                def after_mm_group():
                    mmg[0] += 1
                    if mmg[0] >= 5 and fpos[0] < len(fillers):
                        fillers[fpos[0]]()
                        fpos[0] += 1
                fillers_by_bt["pos", bt + 1] = fpos

                xts = xts_by_bt[bt]
                h2s = {}
                for side in ("u", "v"):
                    xz, xf, xc = (
                        xts[(side, "z")], xts[(side, "f")], xts[(side, "c")],
                    )
                    h1s = {}
                    for br, pairs in (
                        ("z", ((xz, 0, 0), (xz, 2, 2), (xf, 0, 4), (xf, 2, 6))),
                        ("c", ((xf, 0, 4), (xf, 2, 6), (xc, 0, 8), (xc, 2, 10))),
                    ):
                        h1 = hp.tile(
                            [P, DC, BT], fp8, name=f"h1{side}{br}",
                            tag=f"h1{side}{br}",
                        )
                        for m in range(DC):
                            ps = ps_mm.tile([P, BT], f32, name="ps1", tag="mm")
                            for ji, (xt, xo, wo) in enumerate(pairs):
                                nc.tensor.matmul(
                                    ps[:],
                                    WL1_sb[:, wo : wo + 2, m * P : (m + 1) * P],
                                    xt[:, xo : xo + 2, :],
                                    start=(ji == 0),
                                    stop=(ji == len(pairs) - 1),
                                    perf_mode=DR,
                                )
                            drain(h1[:, m, :], ps[:], True)
                            if m % 2 == 1:
                                after_mm_group()
                        h1s[br] = h1

                    for br in ("z", "c"):
                        h2 = hp.tile(
                            [P, DC, BT], bf16, name=f"h2{side}{br}",
                            tag=f"h2{side}{br}",
                        )
                        for m in range(DC):
                            ps = ps_mm.tile([P, BT], f32, name="ps2", tag="mm")
                            for ji in range(2):
                                nc.tensor.matmul(
                                    ps[:],
                                    A2T8_sb[:, 2 * ji : 2 * ji + 2, m * P : (m + 1) * P],
                                    h1s[br][:, 2 * ji : 2 * ji + 2, :],
                                    start=(ji == 0),
                                    stop=(ji == 1),
                                    perf_mode=DR,
                                )
                            drain(h2[:, m, :], ps[:], True)
                            if m % 2 == 1:
                                after_mm_group()
                        h2s[(side, br)] = h2
                h2s_by_bt[bt] = h2s

            def phaseA2(bt):
                """L3 scores + sigmoid + fusion for tile bt."""
                h2s = h2s_by_bt.pop(bt)
                # L3: dT = a3 . (h2z - h2c) as batch-on-partition matvecs
                dps = ps_aux.tile([P, 2, NSUB], f32, name="dps", tag="aux")
                for si, side in enumerate(("u", "v")):
                    for t in range(NSUB):
                        for k in range(DC):
                            nc.tensor.matmul(
                                dps[:, si, t : t + 1],
                                h2s[(side, "z")][:, k, t * P : (t + 1) * P],
                                a3p_sb[:, k : k + 1],
                                start=(k == 0),
                                stop=False,
                            )
                        for k in range(DC):
                            nc.tensor.matmul(
                                dps[:, si, t : t + 1],
                                h2s[(side, "c")][:, k, t * P : (t + 1) * P],
                                a3n_sb[:, k : k + 1],
                                start=False,
                                stop=(k == DC - 1),
                            )
                wz = sp.tile([P, 2, NSUB], f32, name="wz", tag="wz")
                wn = sp.tile([P, 2, NSUB], f32, name="wn", tag="wn")
                nc.scalar.activation(out=wz[:], in_=dps[:], func=AF.Sigmoid)
                nc.scalar.activation(
                    out=wn[:], in_=dps[:], func=AF.Sigmoid, scale=-1.0
                )

                # fusion in raw domain: uf = wz*z + wn*c (z-mult + add on
                # DVE, c-mult on gpsimd in parallel)
                ufs = {}
                for si, side in enumerate(("u", "v")):
                    uf = rawp.tile(
                        [P, NSUB, D], bf16, name=f"uf_{side}", tag=f"uf_{side}"
                    )
                    tmp = rawp.tile(
                        [P, NSUB, D], bf16, name=f"ft_{side}", tag=f"ft_{side}"
                    )
                    raw3 = raws_by_bt[bt][side]
                    for s in range(NSUB):
                        nc.gpsimd.tensor_scalar(
                            out=tmp[:, s, :], in0=raw3[:, 2 * NSUB + s, :],
                            scalar1=wn[:, si, s : s + 1], scalar2=None,
                            op0=ALU.mult,
                        )
                        nc.vector.tensor_scalar(
                            out=uf[:, s, :], in0=raw3[:, 0 * NSUB + s, :],
                            scalar1=wz[:, si, s : s + 1], scalar2=None,
                            op0=ALU.mult,
                        )
                        nc.vector.tensor_tensor(
                            out=uf[:, s, :], in0=uf[:, s, :], in1=tmp[:, s, :],
                            op=ALU.add,
                        )
                    ufs[side] = uf
                ufs_by_bt[bt] = ufs
                # leftover filler transposes cover the fusion window
                held = fillers_by_bt.pop(bt + 1, [])
                fpos = fillers_by_bt.pop(("pos", bt + 1), [0])
                for u_ in held[fpos[0]:]:
                    u_()

            def phaseB(bt):
                """Head phase for tile bt (emitted one iteration later so the
                next tile's L1 covers the sigmoid+fusion latency)."""
                ufs = ufs_by_bt.pop(bt)
                held = []
                hi = [0]
                fusedT = {}
                for side in ("u", "v"):
                    ufT = xp.tile(
                        [P, DC, BT], bf16, name=f"ufT_{side}", tag=f"ufT_{side}"
                    )
                    for half in range(2):
                        transpose_group(
                            lambda s, c, u=ufs[side]: u[:, s, c * P : (c + 1) * P],
                            ufT, half,
                        )
                        if hi[0] < len(held):
                            held[hi[0]]()
                            hi[0] += 1
                    fusedT[side] = ufT
                for u_ in held[hi[0]:]:
                    u_()

                hh = hp.tile([P, DC, BT], bf16, name="hh", tag="hh")
                for m in range(DC):
                    ps = ps_mm.tile([P, BT], f32, name="psh", tag="mm")
                    for k in range(DC):
                        nc.tensor.matmul(
                            ps[:],
                            W1uT_sb[:, k, m * P : (m + 1) * P],
                            fusedT["u"][:, k, :],
                            start=(k == 0),
                            stop=False,
                        )
                    for k in range(DC):
                        nc.tensor.matmul(
                            ps[:],
                            W1vT_sb[:, k, m * P : (m + 1) * P],
                            fusedT["v"][:, k, :],
                            start=False,
                            stop=(k == DC - 1),
                        )
                    drain(hh[:, m, :], ps[:], True)

                ops_ = ps_aux.tile([P, NSUB], f32, name="ops", tag="aux")
                for t in range(NSUB):
                    for k in range(DC):
                        nc.tensor.matmul(
                            ops_[:, t : t + 1],
                            hh[:, k, t * P : (t + 1) * P],
                            w2T_sb[:, k : k + 1],
                            start=(k == 0),
                            stop=(k == DC - 1),
                        )
                osb = sp.tile([P, NSUB], f32, name="osb", tag="osb")
                nc.vector.tensor_copy(osb[:], ops_[:])
                nc.sync.dma_start(
                    out=out[bt * BT : (bt + 1) * BT].rearrange(
                        "(t p) -> p t", p=P
                    ),
                    in_=osb[:],
                )

            for bt in range(NBT):
                phaseA1(bt)
                phaseA2(bt)
                if bt > 0:
                    phaseB(bt - 1)
            phaseB(NBT - 1)

    _split_multi_waits(nc)
    return nc


_NC_CACHE = None


def _get_nc():
    global _NC_CACHE
    if _NC_CACHE is None:
        _NC_CACHE = _build()
    return _NC_CACHE


def _prep_host(inputs):
    """Host-side weight preprocessing shared by all cores."""
    f = lambda k: np.asarray(inputs[k], np.float32)
    att_w1 = f("att_w1")
    att_w2 = f("att_w2")
    w1 = f("w1")
    s = f("bn_gamma") / np.sqrt(f("bn_var") + BN_EPS)
    # all bias-like vectors are structurally zero for this problem (see
    # spec fill types); the device kernel relies on that.
    for k in ("att_b1", "att_b2", "b1", "bn_beta", "bn_mean"):
        assert np.abs(f(k)).max() == 0.0, f"nonzero bias {k}"
    bf = lambda a: np.ascontiguousarray(a).astype(ml_dtypes.bfloat16)
    f8 = lambda a: np.ascontiguousarray(a).astype(ml_dtypes.float8_e4m3)
    A1aT = att_w1[:, :D].T
    A1fT = att_w1[:, D:].T
    common = {
        # z/f/c stacked per side; gather indices are pre-offset per block
        "EU3": bf(np.concatenate([f("Ez_u"), f("E_u"), f("Ec_u")], axis=0)),
        "EV3": bf(np.concatenate([f("Ez_v"), f("E_v"), f("Ec_v")], axis=0)),
        "WL1": f8(np.concatenate([A1aT, A1fT, A1aT], axis=0)),
        "A2T8": f8(att_w2.T),
        "W1uT": bf((w1[:, :D] * s[:, None]).T),
        "W1vT": bf((w1[:, D:] * s[:, None]).T),
        "a3p": bf(f("att_w3")[0]),
        "a3n": bf(-f("att_w3")[0]),
        "w2T": bf(f("w2")[0]),
    }
    return common


def kernel(**inputs):
    common = _prep_host(inputs)
    nodes_u = np.asarray(inputs["nodes_u"]).astype(np.int32)
    nodes_v = np.asarray(inputs["nodes_v"]).astype(np.int32)

    koff_u = np.array([0, NU, 2 * NU], np.int32)
    koff_v = np.array([0, NV, 2 * NV], np.int32)

    def mk_idx3(nodes, koff):
        # [P, 3, T]: [p, k, t] = nodes[t*P+p] + koff[k], flattened to [P, 3T]
        pt = nodes.reshape(-1, P).T  # [P, T]
        arr = pt[:, None, :] + koff[None, :, None]
        return np.ascontiguousarray(arr.reshape(P, -1).astype(np.int32))

    in_maps = []
    for i in range(NCORES):
        m = dict(common)
        m["idx3_u"] = mk_idx3(nodes_u[i * BC : (i + 1) * BC], koff_u)
        m["idx3_v"] = mk_idx3(nodes_v[i * BC : (i + 1) * BC], koff_v)
        in_maps.append(m)

    nc = _get_nc()
    res = run_bass_kernel_spmd(nc, in_maps, core_ids=list(range(NCORES)))
    out = np.concatenate([np.asarray(r["out"]) for r in res.results])
    return (out + np.float32(np.asarray(inputs["b2"]).reshape(-1)[0])).astype(np.float32)
